# revision 23
# baseline (speedup 1.0000x reference)
"""Causal self-attention (B=2, S=2048, D=1024, H=16) on 8 trn2 NeuronCores.

Sharding: batch x head-group. Core c handles batch c//4 and heads
[ (c%4)*4 , (c%4)*4+4 ).  QKV projections are column-sharded, the output
projection row-sharded (Megatron style); each core produces a partial
[S, D] output which the host sums per batch.

v2 strategy (vs the fp32r baseline):
  - everything bf16 on the matmul path (rel-err ~4e-3, well under the
    1e-2 gate); fp8 was measured to break accuracy (peaked self-attention
    means no softmax damping of upstream quantization error).
  - x is uploaded PRE-TRANSPOSED (and d-chunk-major) from the host, so
    the kernel needs no PE transposes and no PSUM->SBUF transpose copies.
  - weights are uploaded pre-chunked bf16, so no on-chip convert pass.
  - softmax normalizer broadcast uses gpsimd partition_broadcast instead
    of a DRAM round-trip.
  - output projection PSUM is DMAd straight to DRAM (no staging copy).

Per-core layout:
  xt    [128, 8, S]   x^T, d-chunk-major (partition p of chunk kc is
                      dmodel index kc*128+p)
  Q^T,K^T [128, 2, S] = W^T x^T  (lhsT = W cols, rhs = x^T)
  V     [128, NT, HPG, VW] = x W, padded with a ones column per head so
        the AV matmul also produces the softmax normalizer l.
  S^T   [k, q] score chunks; exp() applied directly (scores bounded);
        causal mask = skip fully-masked leading columns + one triangular
        multiply on the diagonal 128-block of the exp output.
  out'^T [65, q] = [V|1]^T A^T accumulated over k tiles in PSUM.
  O^T = out'^T[0:64] * (1/l), l-reciprocal broadcast across partitions
        via gpsimd.
  out   [S, D] = O^T^T Wo accumulated over the 2 feature chunks,
        DMAd from PSUM.
"""

import numpy as np
import ml_dtypes

import concourse.bass as bass
import concourse.mybir as mybir
import concourse.tile as tile
from concourse.bass_utils import run_bass_kernel_spmd

B, S, D = 2, 2048, 1024
HPG, DH = 4, 64            # heads per core, head dim
OC = HPG * DH              # 256 projection cols per core
VW = DH + 1                # V padded with ones column
NT = S // 128              # 16 token tiles
NM = D // 128              # 8 dmodel chunks
QC = 512                   # q chunk width
NQC = S // QC              # 4 q chunks
F32 = mybir.dt.float32
BF16 = mybir.dt.bfloat16
NPBF = ml_dtypes.bfloat16

_NC_CACHE = {}


WAIT_CAP = 1


def _split_waits_bir(bir_json, cap=WAIT_CAP):
    """This container's walrus rejects instructions carrying more than `cap`
    sync waits.  Hoist the excess into standalone same-engine EventSemaphore
    wait ops immediately before the instruction (sequencers execute in
    order, so semantics are identical)."""
    import json as _json

    d = _json.loads(bir_json)
    n_split = 0
    for f in d.get("functions", []):
        for bb in f.get("blocks", []):
            insts = bb.get("instructions", [])
            out = []
            for inst in insts:
                si = inst.get("sync_info")
                ow = (si or {}).get("on_wait") or []
                sem_w = [w for w in ow if w.get("sync_type") == "semaphore"]
                other_w = [w for w in ow if w.get("sync_type") != "semaphore"]
                budget = max(cap - len(other_w), 0)
                if len(sem_w) > budget:
                    keep = sem_w[:budget] if budget else []
                    extra = sem_w[budget:]
                    step = max(cap, 1)
                    for i in range(0, len(extra), step):
                        n_split += 1
                        out.append({
                            "debug": inst.get("debug"),
                            "engine": inst["engine"],
                            "ins": [],
                            "name": f"{inst['name']}_sw{i}",
                            "opcode": "EventSemaphore",
                            "outs": [],
                            "sync_info": {"on_update": [],
                                          "on_wait": extra[i:i + step]},
                        })
                    si["on_wait"] = other_w + keep
                out.append(inst)
            bb["instructions"] = out
    return _json.dumps(d).encode(), n_split


def _patch_compile_hook():
    import concourse.bass_utils as bu
    import concourse.bass2jax as b2j

    orig = bu.compile_bir_kernel
    if getattr(orig, "_split_waits_wrapped", False):
        return

    def wrapped(bir_json, tmpdir, neff_name="file.neff"):
        if isinstance(bir_json, str):
            bir_json = bir_json.encode()
        bir_json, _ = _split_waits_bir(bir_json)
        return orig(bir_json, tmpdir, neff_name)

    wrapped._split_waits_wrapped = True
    bu.compile_bir_kernel = wrapped
    if getattr(b2j, "compile_bir_kernel", None) is orig:
        b2j.compile_bir_kernel = wrapped


def _patch_tile_drain():
    """This container's walrus rejects >2 sync waits on one SP CTRL op; the
    stock Tile exit drain carries one wait per active proc.  Emit separate
    single-wait instructions instead."""
    from concourse.vector_clock import ScopedClock  # noqa: F401

    def _drain_split(self, tick_clock, wait_clock):
        nc = self.nc
        sems = wait_clock.sems.allocated()
        for proc, t in enumerate(list(tick_clock.global_clock)):
            if t <= 0:
                continue
            sem = sems.get(proc)
            if sem is None:
                continue
            nc.sync.wait_ge(sem, t * (16 if sem.name.startswith("DMA") else 1))
        nc.sync.drain()
        nc.all_engine_barrier()
        popped = nc._tile_sem_poison_stack.pop()
        assert popped is self._sem_poison
        nc.clear_and_free_semaphores(list(self.sems.allocated().values()))
        nc.all_engine_barrier()

    tile.TileContext._drain_and_barrier = _drain_split


def _bc(ap, n):
    """Broadcast a [1, ...] DRAM AP across n partitions (step-0 partition)."""
    return bass.AP(tensor=ap.tensor, offset=ap.offset, ap=[[0, n]] + list(ap.ap)[1:])


def build_nc():
    nc = bass.Bass()
    xt = nc.dram_tensor("xt", [128, NM, S], BF16, kind="ExternalInput")
    wq = nc.dram_tensor("wq", [128, NM, OC], BF16, kind="ExternalInput")
    wk = nc.dram_tensor("wk", [128, NM, OC], BF16, kind="ExternalInput")
    wv = nc.dram_tensor("wv", [128, NM, OC], BF16, kind="ExternalInput")
    wo = nc.dram_tensor("wo", [128, 2, D], BF16, kind="ExternalInput")
    bq = nc.dram_tensor("bq", [OC], F32, kind="ExternalInput")
    bk = nc.dram_tensor("bk", [OC], F32, kind="ExternalInput")
    bv = nc.dram_tensor("bv", [OC], F32, kind="ExternalInput")
    out = nc.dram_tensor("out", [S, D], F32, kind="ExternalOutput")

    bqr = bq.rearrange("(p one) -> p one", one=1)
    bkr = bk.rearrange("(p one) -> p one", one=1)
    bvr = bv.rearrange("(one c) -> one c", one=1)

    with tile.TileContext(nc) as tc:
        with (
            tc.tile_pool(name="singles", bufs=1) as sing,
            tc.tile_pool(name="persist", bufs=1) as per,
            tc.tile_pool(name="xtg", bufs=4) as xtp,
            tc.tile_pool(name="apool", bufs=6) as apool,
            tc.tile_pool(name="rpool", bufs=2) as rpool,
            tc.tile_pool(name="opool", bufs=3) as opool,
            tc.tile_pool(name="pp", bufs=2, space="PSUM") as pp,
        ):
            # Startup DMA order is latency-critical: the first Q-projection
            # matmul needs wq chunks 0-3 and x^T chunks 0-3, so issue those
            # first in small pieces; everything else follows.
            wq_sb = sing.tile([128, NM, OC], BF16, tag="wq")
            wk_sb = sing.tile([128, NM, OC], BF16, tag="wk")
            wv_sb = sing.tile([128, NM, OC], BF16, tag="wv")
            wo_sb = sing.tile([128, 2, D], BF16, tag="wo")
            xtg = {}
            for g in range(NQC):
                xtg[g] = xtp.tile([128, NM, QC], BF16, tag="xtg",
                                  name=f"xtg{g}")
            # DMA issue order tracks first-use time on the PE
            nc.sync.dma_start(out=wq_sb[:, 0:4, :], in_=wq[:, 0:4, :])
            nc.sync.dma_start(out=xtg[0][:, 0:4, :], in_=xt[:, 0:4, 0:QC])
            nc.sync.dma_start(out=wq_sb[:, 4:8, :], in_=wq[:, 4:8, :])
            nc.sync.dma_start(out=xtg[0][:, 4:8, :], in_=xt[:, 4:8, 0:QC])
            nc.sync.dma_start(out=wk_sb[:, 0:4, :], in_=wk[:, 0:4, :])
            nc.sync.dma_start(out=wk_sb[:, 4:8, :], in_=wk[:, 4:8, :])
            nc.sync.dma_start(out=wv_sb, in_=wv[:, :, :])

            tri = sing.tile([128, 128], BF16, tag="tri")
            nc.vector.memset(tri, 1.0)
            nc.gpsimd.affine_select(
                out=tri, in_=tri, compare_op=mybir.AluOpType.is_ge,
                fill=0.0, base=0, channel_multiplier=-1, pattern=[[1, 128]])
            ones = sing.tile([1, 64], BF16, tag="ones")
            nc.vector.memset(ones, 1.0)

            bq_sb = sing.tile([128, 2], F32, tag="bq")
            bk_sb = sing.tile([128, 2], F32, tag="bk")
            for o in range(2):
                nc.sync.dma_start(out=bq_sb[:, o:o + 1], in_=bqr[o * 128:(o + 1) * 128, :])
                nc.sync.dma_start(out=bk_sb[:, o:o + 1], in_=bkr[o * 128:(o + 1) * 128, :])
            bv_sb = sing.tile([128, OC], F32, tag="bv")
            nc.sync.dma_start(out=bv_sb, in_=_bc(bvr[0:1, :], 128))
            bv4 = bv_sb.rearrange("p (h c) -> p h c", h=HPG)
            # prefetch the remaining x^T groups and wo behind the hot DMAs
            nc.sync.dma_start(out=xtg[1], in_=xt[:, :, QC:2 * QC])
            nc.sync.dma_start(out=wo_sb, in_=wo[:, :, :])
            nc.sync.dma_start(out=xtg[2], in_=xt[:, :, 2 * QC:3 * QC])
            nc.sync.dma_start(out=xtg[3], in_=xt[:, :, 3 * QC:4 * QC])

            qt = per.tile([128, 2, S], BF16, tag="qt", name="qt")
            kt = per.tile([128, 2, S], BF16, tag="kt", name="kt")
            ot = per.tile([128, 2, S], BF16, tag="ot", name="ot")
            v4 = per.tile([128, NT, HPG, VW], BF16, tag="v4", name="v4")

            def interleave(*lists):
                import heapq
                h, outl = [], []
                for li, L in enumerate(lists):
                    if L:
                        heapq.heappush(h, (0.0, li, 0))
                while h:
                    pos, li, idx = heapq.heappop(h)
                    outl.append(lists[li][idx])
                    if idx + 1 < len(lists[li]):
                        heapq.heappush(h, (pos + 1.0 / len(lists[li]), li, idx + 1))
                return outl

            def ab_items(g):
                """Projections for token group g (512 tokens)."""
                items = []
                qk_ps = {}

                def qk_chunk(wsb, bsb, dst, o, half):
                    # Q (or K) for both o-halves shares one [128,1024] psum
                    def f():
                        if half == 0 and o == 0:
                            qk_ps[id(wsb)] = pp.tile(
                                [128, 1024], F32, tag="big", bufs=3,
                                name=f"qk{g}_{o}")
                        ps = qk_ps[id(wsb)][:, o * 512:(o + 1) * 512]
                        for kc in range(4 * half, 4 * half + 4):
                            nc.tensor.matmul(
                                ps,
                                lhsT=wsb[:, kc, o * 128:(o + 1) * 128],
                                rhs=xtg[g][:, kc, :],
                                start=(kc == 0), stop=(kc == NM - 1))
                        if half == 1:
                            nc.vector.tensor_scalar_add(
                                out=dst[:, o, g * QC:(g + 1) * QC],
                                in0=ps, scalar1=bsb[:, o:o + 1])
                    return f
                for wsb, bsb, dst in ((wq_sb, bq_sb, qt), (wk_sb, bk_sb, kt)):
                    for half in range(2):
                        for o in range(2):
                            items.append(qk_chunk(wsb, bsb, dst, o, half))

                v_ps = {}

                def v_chunk(tp):
                    # all four token tiles share one [128, 1024] psum
                    def f():
                        if tp == 0:
                            v_ps[0] = pp.tile([128, 4, OC], F32, tag="big",
                                              bufs=3, name=f"pv{g}")
                        pv = v_ps[0]
                        for i in range(2):
                            tt = 4 * g + tp + i
                            for kc in range(NM):
                                nc.tensor.matmul(
                                    pv[:, tp + i, :],
                                    lhsT=xtg[g][:, kc, (tp + i) * 128:(tp + i + 1) * 128],
                                    rhs=wv_sb[:, kc, :],
                                    start=(kc == 0), stop=(kc == NM - 1))
                        for i in range(2):
                            tt = 4 * g + tp + i
                            nc.vector.tensor_add(
                                out=v4[:, tt, :, 0:DH],
                                in0=pv[:, tp + i, :].rearrange("p (h c) -> p h c", h=HPG),
                                in1=bv4)
                            nc.gpsimd.memset(v4[:, tt, :, DH:VW], 1.0)
                    return f
                for tp in (0, 2):
                    items.append(v_chunk(tp))
                return items

            def c_items(qc):
                """Attention for q chunk qc (512 queries)."""
                items = []
                nkt = 4 * qc + 4
                pavs = {}

                def pair_step(h, ktp):
                    o, r = h // 2, (h % 2) * 64
                    def f():
                        qt_h = qt[r:r + 64, o, :]
                        kt_h = kt[r:r + 64, o, :]
                        if ktp == 0:
                            pavs[h] = pp.tile([VW, QC], F32, tag="pav",
                                              name=f"pav{qc}_{h}")
                        pav = pavs[h]
                        kts = [k for k in (ktp, ktp + 1) if k < nkt]
                        ps = pp.tile([128, 1024], F32, tag="big", bufs=3,
                                     name=f"ps{qc}_{h}_{ktp}")
                        offs = [max(k * 128 - qc * QC, 0) for k in kts]
                        for i, k in enumerate(kts):
                            nc.tensor.matmul(
                                ps[:, i * 512 + offs[i]:(i + 1) * 512],
                                lhsT=kt_h[:, k * 128:(k + 1) * 128],
                                rhs=qt_h[:, qc * QC + offs[i]:(qc + 1) * QC],
                                start=True, stop=True)
                        at = apool.tile([128, 1024], BF16, tag="at",
                                        name=f"at{qc}_{h}_{ktp}")
                        if offs[0] == offs[-1]:
                            # same offset: one exp instruction for the pair
                            w = 512 * len(kts)
                            nc.scalar.activation(
                                out=at[:, offs[0]:w], in_=ps[:, offs[0]:w],
                                func=mybir.ActivationFunctionType.Exp,
                                scale=1.0 / 8.0)
                        else:
                            for i, k in enumerate(kts):
                                nc.scalar.activation(
                                    out=at[:, i * 512 + offs[i]:(i + 1) * 512],
                                    in_=ps[:, i * 512 + offs[i]:(i + 1) * 512],
                                    func=mybir.ActivationFunctionType.Exp,
                                    scale=1.0 / 8.0)
                        for i, k in enumerate(kts):
                            off = offs[i]
                            if k * 128 - qc * QC >= 0:
                                nc.gpsimd.tensor_mul(
                                    out=at[:, i * 512 + off:i * 512 + off + 128],
                                    in0=at[:, i * 512 + off:i * 512 + off + 128],
                                    in1=tri)
                            nc.tensor.matmul(
                                pav[:, off:QC],
                                lhsT=v4[:, k, h, :],
                                rhs=at[:, i * 512 + off:(i + 1) * 512],
                                start=(k == 0), stop=(k == nkt - 1))
                    return f

                recs = {}

                def norm_recip(h):
                    def f():
                        rec = rpool.tile([1, QC], BF16, tag="rec",
                                         name=f"rec{qc}_{h}")
                        with nc.allow_low_precision(
                                reason="1/l broadcast fed through bf16 PE "
                                       "outer product; 0.4% on the "
                                       "normalizer is within budget"):
                            nc.vector.reciprocal(out=rec, in_=pavs[h][DH:VW, :])
                        recs[h] = rec
                    return f

                def norm_apply(h):
                    # PE outer-product broadcast of 1/l; deferred into the
                    # next head's instruction stream so the in-order PE queue
                    # has score work in front of it while DVE computes rec.
                    o, r = h // 2, (h % 2) * 64
                    def f():
                        rbt = pp.tile([128, 1024], F32, tag="big", bufs=3,
                                      name=f"rbp{qc}_{h}")
                        rbp = rbt[0:64, 0:QC]
                        nc.tensor.matmul(rbp, lhsT=ones, rhs=recs[h],
                                         start=True, stop=True)
                        rb = rpool.tile([64, QC], F32, tag="rb",
                                        name=f"rb{qc}_{h}")
                        nc.vector.tensor_copy(out=rb, in_=rbp)
                        nc.vector.tensor_mul(
                            out=ot[r:r + 64, o, qc * QC:(qc + 1) * QC],
                            in0=pavs[h][0:DH, :], in1=rb)
                    return f

                pend = None
                for h in range(HPG):
                    for j, ktp in enumerate(range(0, nkt, 2)):
                        items.append(pair_step(h, ktp))
                        if j == 0 and pend is not None:
                            items.append(pend)
                            pend = None
                    items.append(norm_recip(h))
                    pend = norm_apply(h)
                items.append(pend)
                return items

            def d_items(g, fin=False):
                """Output projection for token tiles 4g..4g+3."""
                items = []

                def out_tile(tt):
                    def f():
                        ob = opool.tile([128, D], F32, tag="ob", name=f"ob{tt}")
                        po = pp.tile([128, 1024], F32, tag="big", bufs=3,
                                     name=f"po{tt}")
                        for nb in range(2):
                            for cb in range(2):
                                nc.tensor.matmul(
                                    po[:, nb * 512:(nb + 1) * 512],
                                    lhsT=ot[:, cb, tt * 128:(tt + 1) * 128],
                                    rhs=wo_sb[:, cb, nb * 512:(nb + 1) * 512],
                                    start=(cb == 0), stop=(cb == 1))
                        if fin:
                            # split copies across ACT and DVE on the last tiles
                            nc.scalar.copy(out=ob[:, 0:512], in_=po[:, 0:512])
                            nc.vector.tensor_copy(out=ob[:, 512:1024],
                                                  in_=po[:, 512:1024])
                        else:
                            nc.vector.tensor_copy(out=ob, in_=po)
                        nc.sync.dma_start(
                            out=out[tt * 128:(tt + 1) * 128, :], in_=ob)
                    return f
                for tt in range(4 * g, 4 * g + 4):
                    items.append(out_tile(tt))
                return items

            # warm up the tensor engine while the first DMAs land: the
            # p-state model needs ~3us of continuous execution to reach
            # full clock, and the first real matmul can't start before the
            # wq/x^T transfers complete (~4us).
            warm = pp.tile([128, 1024], F32, tag="big", bufs=3, name="warm")
            for i in range(30):
                nc.tensor.matmul(warm[:, 0:128], lhsT=tri, rhs=tri,
                                 start=True, stop=True)

            # round 0: group 0 projections alone
            for f in ab_items(0):
                f()
            # round 1: attention(0) + projections(1)
            for f in interleave(c_items(0), ab_items(1)):
                f()
            # rounds 2,3: attention(r-1) + projections(r) + outproj(r-2)
            for r in (2, 3):
                for f in interleave(c_items(r - 1), ab_items(r) + d_items(r - 2)):
                    f()
            # final attention chunk + remaining output projections
            for f in interleave(c_items(NQC - 1), d_items(2)):
                f()
            for f in d_items(NQC - 1, fin=True):
                f()
    return nc


def _get_nc():
    key = "v2"
    if key not in _NC_CACHE:
        _patch_tile_drain()
        _patch_compile_hook()
        _NC_CACHE[key] = build_nc()
    return _NC_CACHE[key]


def make_in_maps(inputs):
    x = np.asarray(inputs["x"], dtype=np.float32)
    Wq = np.asarray(inputs["Wq"], dtype=np.float32)
    Wk = np.asarray(inputs["Wk"], dtype=np.float32)
    Wv = np.asarray(inputs["Wv"], dtype=np.float32)
    Wo = np.asarray(inputs["Wo"], dtype=np.float32)
    bq = np.asarray(inputs["bq"], dtype=np.float32)
    bk = np.asarray(inputs["bk"], dtype=np.float32)
    bv = np.asarray(inputs["bv"], dtype=np.float32)

    def chunked_w(w):
        # [D, OC] -> [128, NM, OC] bf16, d-chunk-major on partitions
        return np.ascontiguousarray(
            w.reshape(NM, 128, -1).transpose(1, 0, 2).astype(NPBF))

    xts = []
    for b in range(B):
        xts.append(np.ascontiguousarray(
            x[b].T.reshape(NM, 128, S).transpose(1, 0, 2).astype(NPBF)))

    in_maps = []
    for c in range(8):
        b, g = c // 4, c % 4
        cols = slice(g * OC, (g + 1) * OC)
        in_maps.append({
            "xt": xts[b],
            "wq": chunked_w(Wq[:, cols]),
            "wk": chunked_w(Wk[:, cols]),
            "wv": chunked_w(Wv[:, cols]),
            "wo": np.ascontiguousarray(
                Wo[cols, :].reshape(2, 128, D).transpose(1, 0, 2).astype(NPBF)),
            "bq": np.ascontiguousarray(bq[cols]),
            "bk": np.ascontiguousarray(bk[cols]),
            "bv": np.ascontiguousarray(bv[cols]),
        })
    return in_maps


def combine(results, inputs):
    bo = np.asarray(inputs["bo"], dtype=np.float32)
    out = np.zeros((B, S, D), dtype=np.float32)
    for c in range(8):
        out[c // 4] += np.asarray(results[c]["out"], dtype=np.float32)
    out += bo[None, None, :]
    return out


def kernel(**inputs) -> np.ndarray:
    nc = _get_nc()
    in_maps = make_in_maps(inputs)
    res = run_bass_kernel_spmd(nc, in_maps, core_ids=list(range(8)))
    return combine(res.results, inputs)


if __name__ == "__main__":
    import jax
    print(jax.devices())


# revision 31
# speedup vs baseline: 1.0684x; 1.0684x over previous
"""Causal self-attention (B=2, S=2048, D=1024, H=16) on 8 trn2 NeuronCores.

Sharding: batch x head-group. Core c handles batch c//4 and heads
[ (c%4)*4 , (c%4)*4+4 ).  QKV projections are column-sharded, the output
projection row-sharded (Megatron style); each core produces a partial
[S, D] output which the host sums per batch.

v2 strategy (vs the fp32r baseline):
  - everything bf16 on the matmul path (rel-err ~4e-3, well under the
    1e-2 gate); fp8 was measured to break accuracy (peaked self-attention
    means no softmax damping of upstream quantization error).
  - x is uploaded PRE-TRANSPOSED (and d-chunk-major) from the host, so
    the kernel needs no PE transposes and no PSUM->SBUF transpose copies.
  - weights are uploaded pre-chunked bf16, so no on-chip convert pass.
  - softmax normalizer broadcast uses gpsimd partition_broadcast instead
    of a DRAM round-trip.
  - output projection PSUM is DMAd straight to DRAM (no staging copy).

Per-core layout:
  xt    [128, 8, S]   x^T, d-chunk-major (partition p of chunk kc is
                      dmodel index kc*128+p)
  Q^T,K^T [128, 2, S] = W^T x^T  (lhsT = W cols, rhs = x^T)
  V     [128, NT, HPG, VW] = x W, padded with a ones column per head so
        the AV matmul also produces the softmax normalizer l.
  S^T   [k, q] score chunks; exp() applied directly (scores bounded);
        causal mask = skip fully-masked leading columns + one triangular
        multiply on the diagonal 128-block of the exp output.
  out'^T [65, q] = [V|1]^T A^T accumulated over k tiles in PSUM.
  O^T = out'^T[0:64] * (1/l), l-reciprocal broadcast across partitions
        via gpsimd.
  out   [S, D] = O^T^T Wo accumulated over the 2 feature chunks,
        DMAd from PSUM.
"""

import numpy as np
import ml_dtypes

import concourse.bass as bass
import concourse.mybir as mybir
import concourse.tile as tile
from concourse.bass_utils import run_bass_kernel_spmd

B, S, D = 2, 2048, 1024
HPG, DH = 4, 64            # heads per core, head dim
OC = HPG * DH              # 256 projection cols per core
VW = DH + 1                # V padded with ones column
NT = S // 128              # 16 token tiles
NM = D // 128              # 8 dmodel chunks
QC = 512                   # q chunk width
NQC = S // QC              # 4 q chunks
F32 = mybir.dt.float32
BF16 = mybir.dt.bfloat16
NPBF = ml_dtypes.bfloat16

_NC_CACHE = {}


WAIT_CAP = 1


def _split_waits_bir(bir_json, cap=WAIT_CAP):
    """This container's walrus rejects instructions carrying more than `cap`
    sync waits.  Hoist the excess into standalone same-engine EventSemaphore
    wait ops immediately before the instruction (sequencers execute in
    order, so semantics are identical)."""
    import json as _json

    d = _json.loads(bir_json)
    n_split = 0
    for f in d.get("functions", []):
        for bb in f.get("blocks", []):
            insts = bb.get("instructions", [])
            out = []
            for inst in insts:
                si = inst.get("sync_info")
                ow = (si or {}).get("on_wait") or []
                sem_w = [w for w in ow if w.get("sync_type") == "semaphore"]
                other_w = [w for w in ow if w.get("sync_type") != "semaphore"]
                budget = max(cap - len(other_w), 0)
                if len(sem_w) > budget:
                    keep = sem_w[:budget] if budget else []
                    extra = sem_w[budget:]
                    step = max(cap, 1)
                    for i in range(0, len(extra), step):
                        n_split += 1
                        out.append({
                            "debug": inst.get("debug"),
                            "engine": inst["engine"],
                            "ins": [],
                            "name": f"{inst['name']}_sw{i}",
                            "opcode": "EventSemaphore",
                            "outs": [],
                            "sync_info": {"on_update": [],
                                          "on_wait": extra[i:i + step]},
                        })
                    si["on_wait"] = other_w + keep
                out.append(inst)
            bb["instructions"] = out
    return _json.dumps(d).encode(), n_split


def _patch_compile_hook():
    import concourse.bass_utils as bu
    import concourse.bass2jax as b2j

    orig = bu.compile_bir_kernel
    if getattr(orig, "_split_waits_wrapped", False):
        return

    def wrapped(bir_json, tmpdir, neff_name="file.neff"):
        if isinstance(bir_json, str):
            bir_json = bir_json.encode()
        bir_json, _ = _split_waits_bir(bir_json)
        return orig(bir_json, tmpdir, neff_name)

    wrapped._split_waits_wrapped = True
    bu.compile_bir_kernel = wrapped
    if getattr(b2j, "compile_bir_kernel", None) is orig:
        b2j.compile_bir_kernel = wrapped


def _patch_tile_drain():
    """This container's walrus rejects >2 sync waits on one SP CTRL op; the
    stock Tile exit drain carries one wait per active proc.  Emit separate
    single-wait instructions instead."""
    from concourse.vector_clock import ScopedClock  # noqa: F401

    def _drain_split(self, tick_clock, wait_clock):
        nc = self.nc
        sems = wait_clock.sems.allocated()
        for proc, t in enumerate(list(tick_clock.global_clock)):
            if t <= 0:
                continue
            sem = sems.get(proc)
            if sem is None:
                continue
            nc.sync.wait_ge(sem, t * (16 if sem.name.startswith("DMA") else 1))
        nc.sync.drain()
        nc.all_engine_barrier()
        popped = nc._tile_sem_poison_stack.pop()
        assert popped is self._sem_poison
        nc.clear_and_free_semaphores(list(self.sems.allocated().values()))
        nc.all_engine_barrier()

    tile.TileContext._drain_and_barrier = _drain_split


def _bc(ap, n):
    """Broadcast a [1, ...] DRAM AP across n partitions (step-0 partition)."""
    return bass.AP(tensor=ap.tensor, offset=ap.offset, ap=[[0, n]] + list(ap.ap)[1:])


def build_nc():
    nc = bass.Bass()
    xt = nc.dram_tensor("xt", [128, NM, S], BF16, kind="ExternalInput")
    wq = nc.dram_tensor("wq", [128, NM, OC], BF16, kind="ExternalInput")
    wk = nc.dram_tensor("wk", [128, NM, OC], BF16, kind="ExternalInput")
    wv = nc.dram_tensor("wv", [128, NM, OC], BF16, kind="ExternalInput")
    wo = nc.dram_tensor("wo", [128, 2, D], BF16, kind="ExternalInput")
    bq = nc.dram_tensor("bq", [OC], F32, kind="ExternalInput")
    bk = nc.dram_tensor("bk", [OC], F32, kind="ExternalInput")
    bv = nc.dram_tensor("bv", [OC], F32, kind="ExternalInput")
    out = nc.dram_tensor("out", [S, D], F32, kind="ExternalOutput")

    bqr = bq.rearrange("(p one) -> p one", one=1)
    bkr = bk.rearrange("(p one) -> p one", one=1)
    bvr = bv.rearrange("(one c) -> one c", one=1)

    with tile.TileContext(nc) as tc:
        with (
            tc.tile_pool(name="singles", bufs=1) as sing,
            tc.tile_pool(name="persist", bufs=1) as per,
            tc.tile_pool(name="xtg", bufs=4) as xtp,
            tc.tile_pool(name="apool", bufs=6) as apool,
            tc.tile_pool(name="rpool", bufs=2) as rpool,
            tc.tile_pool(name="opool", bufs=3) as opool,
            tc.tile_pool(name="pp", bufs=2, space="PSUM") as pp,
        ):
            # Startup DMA order is latency-critical: the first Q-projection
            # matmul needs wq chunks 0-3 and x^T chunks 0-3, so issue those
            # first in small pieces; everything else follows.
            wq_sb = sing.tile([128, NM, OC], BF16, tag="wq")
            wk_sb = sing.tile([128, NM, OC], BF16, tag="wk")
            wv_sb = sing.tile([128, NM, OC], BF16, tag="wv")
            wo_sb = sing.tile([128, 2, D], BF16, tag="wo")
            xtg = {}
            for g in range(NQC):
                xtg[g] = xtp.tile([128, NM, QC], BF16, tag="xtg",
                                  name=f"xtg{g}")
            # DMA issue order tracks first-use time on the PE
            nc.sync.dma_start(out=wq_sb[:, 0:4, :], in_=wq[:, 0:4, :])
            nc.sync.dma_start(out=xtg[0][:, 0:4, :], in_=xt[:, 0:4, 0:QC])
            nc.sync.dma_start(out=wq_sb[:, 4:8, :], in_=wq[:, 4:8, :])
            nc.sync.dma_start(out=xtg[0][:, 4:8, :], in_=xt[:, 4:8, 0:QC])
            nc.sync.dma_start(out=wk_sb[:, 0:4, :], in_=wk[:, 0:4, :])
            nc.sync.dma_start(out=wk_sb[:, 4:8, :], in_=wk[:, 4:8, :])
            nc.sync.dma_start(out=wv_sb, in_=wv[:, :, :])

            tri = sing.tile([128, 128], BF16, tag="tri")
            nc.vector.memset(tri, 1.0)
            nc.gpsimd.affine_select(
                out=tri, in_=tri, compare_op=mybir.AluOpType.is_ge,
                fill=0.0, base=0, channel_multiplier=-1, pattern=[[1, 128]])
            ones = sing.tile([1, 64], BF16, tag="ones")
            nc.vector.memset(ones, 1.0)

            bq_sb = sing.tile([128, 2], F32, tag="bq")
            bk_sb = sing.tile([128, 2], F32, tag="bk")
            for o in range(2):
                nc.sync.dma_start(out=bq_sb[:, o:o + 1], in_=bqr[o * 128:(o + 1) * 128, :])
                nc.sync.dma_start(out=bk_sb[:, o:o + 1], in_=bkr[o * 128:(o + 1) * 128, :])
            bv_sb = sing.tile([128, OC], F32, tag="bv")
            nc.sync.dma_start(out=bv_sb, in_=_bc(bvr[0:1, :], 128))
            bv4 = bv_sb.rearrange("p (h c) -> p h c", h=HPG)
            # prefetch the remaining x^T groups and wo behind the hot DMAs
            nc.sync.dma_start(out=xtg[1], in_=xt[:, :, QC:2 * QC])
            nc.sync.dma_start(out=wo_sb, in_=wo[:, :, :])
            nc.sync.dma_start(out=xtg[2], in_=xt[:, :, 2 * QC:3 * QC])
            nc.sync.dma_start(out=xtg[3], in_=xt[:, :, 3 * QC:4 * QC])

            qt = per.tile([128, 2, S], BF16, tag="qt", name="qt")
            kt = per.tile([128, 2, S], BF16, tag="kt", name="kt")
            ot = per.tile([128, 2, S], BF16, tag="ot", name="ot")
            v4 = per.tile([128, NT, HPG, VW], BF16, tag="v4", name="v4")

            def interleave(*lists):
                import heapq
                h, outl = [], []
                for li, L in enumerate(lists):
                    if L:
                        heapq.heappush(h, (0.0, li, 0))
                while h:
                    pos, li, idx = heapq.heappop(h)
                    outl.append(lists[li][idx])
                    if idx + 1 < len(lists[li]):
                        heapq.heappush(h, (pos + 1.0 / len(lists[li]), li, idx + 1))
                return outl

            def ab_items(g):
                """Projections for token group g (512 tokens)."""
                items = []
                qk_ps = {}

                def qk_chunk(wsb, bsb, dst, o, half):
                    def f():
                        if half == 0:
                            qk_ps[(id(wsb), o)] = pp.tile(
                                [128, QC], F32, tag="gp", name=f"qk{g}_{o}")
                        ps = qk_ps[(id(wsb), o)]
                        for kc in range(4 * half, 4 * half + 4):
                            nc.tensor.matmul(
                                ps,
                                lhsT=wsb[:, kc, o * 128:(o + 1) * 128],
                                rhs=xtg[g][:, kc, :],
                                start=(kc == 0), stop=(kc == NM - 1))
                        if half == 1:
                            nc.vector.tensor_scalar_add(
                                out=dst[:, o, g * QC:(g + 1) * QC],
                                in0=ps, scalar1=bsb[:, o:o + 1])
                    return f
                for wsb, bsb, dst in ((wq_sb, bq_sb, qt), (wk_sb, bk_sb, kt)):
                    for half in range(2):
                        for o in range(2):
                            items.append(qk_chunk(wsb, bsb, dst, o, half))

                def v_chunk(tp):
                    # two token tiles (tp, tp+1) share one [128, 512] psum
                    def f():
                        pv = pp.tile([128, 2, OC], F32, tag="gp", name=f"pv{tp}")
                        for i in range(2):
                            tt = 4 * g + tp + i
                            for kc in range(NM):
                                nc.tensor.matmul(
                                    pv[:, i, :],
                                    lhsT=xtg[g][:, kc, (tp + i) * 128:(tp + i + 1) * 128],
                                    rhs=wv_sb[:, kc, :],
                                    start=(kc == 0), stop=(kc == NM - 1))
                        for i in range(2):
                            tt = 4 * g + tp + i
                            nc.vector.tensor_add(
                                out=v4[:, tt, :, 0:DH],
                                in0=pv[:, i, :].rearrange("p (h c) -> p h c", h=HPG),
                                in1=bv4)
                            nc.gpsimd.memset(v4[:, tt, :, DH:VW], 1.0)
                    return f
                for tp in (0, 2):
                    items.append(v_chunk(tp))
                return items

            c_state = {}

            def c_items(qc, heads=tuple(range(HPG))):
                """Attention for q chunk qc (512 queries), given heads."""
                items = []
                nkt = 4 * qc + 4
                pavs = c_state.setdefault(qc, {})

                def pair_step(h, ktp):
                    o, r = h // 2, (h % 2) * 64
                    def f():
                        qt_h = qt[r:r + 64, o, :]
                        kt_h = kt[r:r + 64, o, :]
                        if ktp == 0:
                            pavs[h] = pp.tile([VW, QC], F32, tag="pav",
                                              name=f"pav{qc}_{h}")
                        pav = pavs[h]
                        kts = [k for k in (ktp, ktp + 1) if k < nkt]
                        ps = pp.tile([128, 1024], F32, tag="ps",
                                     name=f"ps{qc}_{h}_{ktp}")
                        offs = [max(k * 128 - qc * QC, 0) for k in kts]
                        for i, k in enumerate(kts):
                            nc.tensor.matmul(
                                ps[:, i * 512 + offs[i]:(i + 1) * 512],
                                lhsT=kt_h[:, k * 128:(k + 1) * 128],
                                rhs=qt_h[:, qc * QC + offs[i]:(qc + 1) * QC],
                                start=True, stop=True)
                        at = apool.tile([128, 1024], BF16, tag="at",
                                        name=f"at{qc}_{h}_{ktp}")
                        if offs[0] == offs[-1]:
                            # same offset: one exp instruction for the pair
                            w = 512 * len(kts)
                            nc.scalar.activation(
                                out=at[:, offs[0]:w], in_=ps[:, offs[0]:w],
                                func=mybir.ActivationFunctionType.Exp,
                                scale=1.0 / 8.0)
                        else:
                            for i, k in enumerate(kts):
                                nc.scalar.activation(
                                    out=at[:, i * 512 + offs[i]:(i + 1) * 512],
                                    in_=ps[:, i * 512 + offs[i]:(i + 1) * 512],
                                    func=mybir.ActivationFunctionType.Exp,
                                    scale=1.0 / 8.0)
                        for i, k in enumerate(kts):
                            off = offs[i]
                            if k * 128 - qc * QC >= 0:
                                nc.gpsimd.tensor_mul(
                                    out=at[:, i * 512 + off:i * 512 + off + 128],
                                    in0=at[:, i * 512 + off:i * 512 + off + 128],
                                    in1=tri)
                            nc.tensor.matmul(
                                pav[:, off:QC],
                                lhsT=v4[:, k, h, :],
                                rhs=at[:, i * 512 + off:(i + 1) * 512],
                                start=(k == 0), stop=(k == nkt - 1))
                    return f

                recs = {}

                def norm_recip(h):
                    def f():
                        rec = rpool.tile([1, QC], BF16, tag="rec",
                                         name=f"rec{qc}_{h}")
                        with nc.allow_low_precision(
                                reason="1/l broadcast fed through bf16 PE "
                                       "outer product; 0.4% on the "
                                       "normalizer is within budget"):
                            nc.vector.reciprocal(out=rec, in_=pavs[h][DH:VW, :])
                        recs[h] = rec
                    return f

                def norm_apply(h):
                    # PE outer-product broadcast of 1/l; deferred into the
                    # next head's instruction stream so the in-order PE queue
                    # has score work in front of it while DVE computes rec.
                    o, r = h // 2, (h % 2) * 64
                    def f():
                        rbt = pp.tile([128, QC], F32, tag="gp",
                                      name=f"rbp{qc}_{h}")
                        rbp = rbt[0:64, :]
                        nc.tensor.matmul(rbp, lhsT=ones, rhs=recs[h],
                                         start=True, stop=True)
                        rb = rpool.tile([64, QC], F32, tag="rb",
                                        name=f"rb{qc}_{h}")
                        nc.vector.tensor_copy(out=rb, in_=rbp)
                        nc.vector.tensor_mul(
                            out=ot[r:r + 64, o, qc * QC:(qc + 1) * QC],
                            in0=pavs[h][0:DH, :], in1=rb)
                    return f

                pend = None
                for h in heads:
                    for j, ktp in enumerate(range(0, nkt, 2)):
                        items.append(pair_step(h, ktp))
                        if j == 0 and pend is not None:
                            items.append(pend)
                            pend = None
                    items.append(norm_recip(h))
                    pend = norm_apply(h)
                items.append(pend)
                return items

            def d_items(g, fin=False):
                """Output projection for token tiles 4g..4g+3."""
                items = []

                def out_tile(tt):
                    def f():
                        ob = opool.tile([128, D], F32, tag="ob", name=f"ob{tt}")
                        for nb in range(2):
                            po = pp.tile([128, 512], F32, tag="gp",
                                         name=f"po{tt}_{nb}")
                            for cb in range(2):
                                nc.tensor.matmul(
                                    po,
                                    lhsT=ot[:, cb, tt * 128:(tt + 1) * 128],
                                    rhs=wo_sb[:, cb, nb * 512:(nb + 1) * 512],
                                    start=(cb == 0), stop=(cb == 1))
                            dst = ob[:, nb * 512:(nb + 1) * 512]
                            if fin and (tt + nb) % 2 == 1:
                                nc.scalar.copy(out=dst, in_=po)
                            else:
                                nc.vector.tensor_copy(out=dst, in_=po)
                            nc.sync.dma_start(
                                out=out[tt * 128:(tt + 1) * 128,
                                        nb * 512:(nb + 1) * 512],
                                in_=dst)
                    return f
                for tt in range(4 * g, 4 * g + 4):
                    items.append(out_tile(tt))
                return items

            # warm up the tensor engine while the first DMAs land: the
            # p-state model needs ~3us of continuous execution to reach
            # full clock, and the first real matmul can't start before the
            # wq/x^T transfers complete (~4us).
            warm = pp.tile([128, 1024], F32, tag="ps", name="warm")
            for i in range(30):
                nc.tensor.matmul(warm[:, 0:128], lhsT=tri, rhs=tri,
                                 start=True, stop=True)

            # Rounds are balanced so per-round ACT (exp) work stays below
            # per-round PE work; qc=3's first two heads are pulled into
            # round 3 because round 4 would otherwise be exp-bound.
            for f in ab_items(0):
                f()
            for f in interleave(c_items(0), ab_items(1)):
                f()
            for f in interleave(c_items(1), ab_items(2) + d_items(0)):
                f()
            for f in interleave(c_items(2), ab_items(3)):
                f()
            for f in interleave(c_items(3, heads=(0, 1)), d_items(1)):
                f()
            for f in interleave(c_items(3, heads=(2, 3)), d_items(2)):
                f()
            for f in d_items(NQC - 1, fin=True):
                f()
    return nc


def _get_nc():
    key = "v2"
    if key not in _NC_CACHE:
        _patch_tile_drain()
        _patch_compile_hook()
        _NC_CACHE[key] = build_nc()
    return _NC_CACHE[key]


def make_in_maps(inputs):
    x = np.asarray(inputs["x"], dtype=np.float32)
    Wq = np.asarray(inputs["Wq"], dtype=np.float32)
    Wk = np.asarray(inputs["Wk"], dtype=np.float32)
    Wv = np.asarray(inputs["Wv"], dtype=np.float32)
    Wo = np.asarray(inputs["Wo"], dtype=np.float32)
    bq = np.asarray(inputs["bq"], dtype=np.float32)
    bk = np.asarray(inputs["bk"], dtype=np.float32)
    bv = np.asarray(inputs["bv"], dtype=np.float32)

    def chunked_w(w):
        # [D, OC] -> [128, NM, OC] bf16, d-chunk-major on partitions
        return np.ascontiguousarray(
            w.reshape(NM, 128, -1).transpose(1, 0, 2).astype(NPBF))

    xts = []
    for b in range(B):
        xts.append(np.ascontiguousarray(
            x[b].T.reshape(NM, 128, S).transpose(1, 0, 2).astype(NPBF)))

    in_maps = []
    for c in range(8):
        b, g = c // 4, c % 4
        cols = slice(g * OC, (g + 1) * OC)
        in_maps.append({
            "xt": xts[b],
            "wq": chunked_w(Wq[:, cols]),
            "wk": chunked_w(Wk[:, cols]),
            "wv": chunked_w(Wv[:, cols]),
            "wo": np.ascontiguousarray(
                Wo[cols, :].reshape(2, 128, D).transpose(1, 0, 2).astype(NPBF)),
            "bq": np.ascontiguousarray(bq[cols]),
            "bk": np.ascontiguousarray(bk[cols]),
            "bv": np.ascontiguousarray(bv[cols]),
        })
    return in_maps


def combine(results, inputs):
    bo = np.asarray(inputs["bo"], dtype=np.float32)
    out = np.zeros((B, S, D), dtype=np.float32)
    for c in range(8):
        out[c // 4] += np.asarray(results[c]["out"], dtype=np.float32)
    out += bo[None, None, :]
    return out


def kernel(**inputs) -> np.ndarray:
    nc = _get_nc()
    in_maps = make_in_maps(inputs)
    res = run_bass_kernel_spmd(nc, in_maps, core_ids=list(range(8)))
    return combine(res.results, inputs)


if __name__ == "__main__":
    import jax
    print(jax.devices())


# revision 34
# speedup vs baseline: 1.1036x; 1.0329x over previous
"""Causal self-attention (B=2, S=2048, D=1024, H=16) on 8 trn2 NeuronCores.

Sharding: batch x head-group. Core c handles batch c//4 and heads
[ (c%4)*4 , (c%4)*4+4 ).  QKV projections are column-sharded, the output
projection row-sharded (Megatron style); each core produces a partial
[S, D] output which the host sums per batch.

v2 strategy (vs the fp32r baseline):
  - everything bf16 on the matmul path (rel-err ~4e-3, well under the
    1e-2 gate); fp8 was measured to break accuracy (peaked self-attention
    means no softmax damping of upstream quantization error).
  - x is uploaded PRE-TRANSPOSED (and d-chunk-major) from the host, so
    the kernel needs no PE transposes and no PSUM->SBUF transpose copies.
  - weights are uploaded pre-chunked bf16, so no on-chip convert pass.
  - softmax normalizer broadcast uses gpsimd partition_broadcast instead
    of a DRAM round-trip.
  - output projection PSUM is DMAd straight to DRAM (no staging copy).

Per-core layout:
  xt    [128, 8, S]   x^T, d-chunk-major (partition p of chunk kc is
                      dmodel index kc*128+p)
  Q^T,K^T [128, 2, S] = W^T x^T  (lhsT = W cols, rhs = x^T)
  V     [128, NT, HPG, VW] = x W, padded with a ones column per head so
        the AV matmul also produces the softmax normalizer l.
  S^T   [k, q] score chunks; exp() applied directly (scores bounded);
        causal mask = skip fully-masked leading columns + one triangular
        multiply on the diagonal 128-block of the exp output.
  out'^T [65, q] = [V|1]^T A^T accumulated over k tiles in PSUM.
  O^T = out'^T[0:64] * (1/l), l-reciprocal broadcast across partitions
        via gpsimd.
  out   [S, D] = O^T^T Wo accumulated over the 2 feature chunks,
        DMAd from PSUM.
"""

import numpy as np
import ml_dtypes

import concourse.bass as bass
import concourse.mybir as mybir
import concourse.tile as tile
from concourse.bass_utils import run_bass_kernel_spmd

B, S, D = 2, 2048, 1024
HPG, DH = 4, 64            # heads per core, head dim
OC = HPG * DH              # 256 projection cols per core
VW = DH + 1                # V padded with ones column
NT = S // 128              # 16 token tiles
NM = D // 128              # 8 dmodel chunks
QC = 512                   # q chunk width
NQC = S // QC              # 4 q chunks
F32 = mybir.dt.float32
BF16 = mybir.dt.bfloat16
NPBF = ml_dtypes.bfloat16

_NC_CACHE = {}


WAIT_CAP = 1


def _split_waits_bir(bir_json, cap=WAIT_CAP):
    """This container's walrus rejects instructions carrying more than `cap`
    sync waits.  Hoist the excess into standalone same-engine EventSemaphore
    wait ops immediately before the instruction (sequencers execute in
    order, so semantics are identical)."""
    import json as _json

    d = _json.loads(bir_json)
    n_split = 0
    for f in d.get("functions", []):
        for bb in f.get("blocks", []):
            insts = bb.get("instructions", [])
            out = []
            for inst in insts:
                si = inst.get("sync_info")
                ow = (si or {}).get("on_wait") or []
                sem_w = [w for w in ow if w.get("sync_type") == "semaphore"]
                other_w = [w for w in ow if w.get("sync_type") != "semaphore"]
                budget = max(cap - len(other_w), 0)
                if len(sem_w) > budget:
                    keep = sem_w[:budget] if budget else []
                    extra = sem_w[budget:]
                    step = max(cap, 1)
                    for i in range(0, len(extra), step):
                        n_split += 1
                        out.append({
                            "debug": inst.get("debug"),
                            "engine": inst["engine"],
                            "ins": [],
                            "name": f"{inst['name']}_sw{i}",
                            "opcode": "EventSemaphore",
                            "outs": [],
                            "sync_info": {"on_update": [],
                                          "on_wait": extra[i:i + step]},
                        })
                    si["on_wait"] = other_w + keep
                out.append(inst)
            bb["instructions"] = out
    return _json.dumps(d).encode(), n_split


def _patch_compile_hook():
    import concourse.bass_utils as bu
    import concourse.bass2jax as b2j

    orig = bu.compile_bir_kernel
    if getattr(orig, "_split_waits_wrapped", False):
        return

    def wrapped(bir_json, tmpdir, neff_name="file.neff"):
        if isinstance(bir_json, str):
            bir_json = bir_json.encode()
        bir_json, _ = _split_waits_bir(bir_json)
        return orig(bir_json, tmpdir, neff_name)

    wrapped._split_waits_wrapped = True
    bu.compile_bir_kernel = wrapped
    if getattr(b2j, "compile_bir_kernel", None) is orig:
        b2j.compile_bir_kernel = wrapped


def _patch_tile_drain():
    """This container's walrus rejects >2 sync waits on one SP CTRL op; the
    stock Tile exit drain carries one wait per active proc.  Emit separate
    single-wait instructions instead."""
    from concourse.vector_clock import ScopedClock  # noqa: F401

    def _drain_split(self, tick_clock, wait_clock):
        nc = self.nc
        sems = wait_clock.sems.allocated()
        for proc, t in enumerate(list(tick_clock.global_clock)):
            if t <= 0:
                continue
            sem = sems.get(proc)
            if sem is None:
                continue
            nc.sync.wait_ge(sem, t * (16 if sem.name.startswith("DMA") else 1))
        nc.sync.drain()
        nc.all_engine_barrier()
        popped = nc._tile_sem_poison_stack.pop()
        assert popped is self._sem_poison
        nc.clear_and_free_semaphores(list(self.sems.allocated().values()))
        nc.all_engine_barrier()

    tile.TileContext._drain_and_barrier = _drain_split


def _bc(ap, n):
    """Broadcast a [1, ...] DRAM AP across n partitions (step-0 partition)."""
    return bass.AP(tensor=ap.tensor, offset=ap.offset, ap=[[0, n]] + list(ap.ap)[1:])


def build_nc():
    nc = bass.Bass()
    xt = nc.dram_tensor("xt", [128, NM, S], BF16, kind="ExternalInput")
    wq = nc.dram_tensor("wq", [128, NM, OC], BF16, kind="ExternalInput")
    wk = nc.dram_tensor("wk", [128, NM, OC], BF16, kind="ExternalInput")
    wv = nc.dram_tensor("wv", [128, NM, OC], BF16, kind="ExternalInput")
    wo = nc.dram_tensor("wo", [128, 2, D], BF16, kind="ExternalInput")
    bq = nc.dram_tensor("bq", [OC], F32, kind="ExternalInput")
    bk = nc.dram_tensor("bk", [OC], F32, kind="ExternalInput")
    bv = nc.dram_tensor("bv", [OC], F32, kind="ExternalInput")
    out = nc.dram_tensor("out", [S, D], F32, kind="ExternalOutput")

    bqr = bq.rearrange("(p one) -> p one", one=1)
    bkr = bk.rearrange("(p one) -> p one", one=1)
    bvr = bv.rearrange("(one c) -> one c", one=1)

    with tile.TileContext(nc) as tc:
        with (
            tc.tile_pool(name="singles", bufs=1) as sing,
            tc.tile_pool(name="persist", bufs=1) as per,
            tc.tile_pool(name="xtg", bufs=4) as xtp,
            tc.tile_pool(name="apool", bufs=6) as apool,
            tc.tile_pool(name="rpool", bufs=2) as rpool,
            tc.tile_pool(name="opool", bufs=3) as opool,
            tc.tile_pool(name="pp", bufs=2, space="PSUM") as pp,
        ):
            # Startup DMA order is latency-critical: the first Q-projection
            # matmul needs wq chunks 0-3 and x^T chunks 0-3, so issue those
            # first in small pieces; everything else follows.
            wq_sb = sing.tile([128, NM, OC], BF16, tag="wq")
            wk_sb = sing.tile([128, NM, OC], BF16, tag="wk")
            wv_sb = sing.tile([128, NM, OC], BF16, tag="wv")
            wo_sb = sing.tile([128, 2, D], BF16, tag="wo")
            xtg = {}
            for g in range(NQC):
                xtg[g] = xtp.tile([128, NM, QC], BF16, tag="xtg",
                                  name=f"xtg{g}")
            # DMA issue order tracks first-use time on the PE
            nc.sync.dma_start(out=wq_sb[:, 0:4, :], in_=wq[:, 0:4, :])
            nc.sync.dma_start(out=xtg[0][:, 0:4, :], in_=xt[:, 0:4, 0:QC])
            nc.sync.dma_start(out=wq_sb[:, 4:8, :], in_=wq[:, 4:8, :])
            nc.sync.dma_start(out=xtg[0][:, 4:8, :], in_=xt[:, 4:8, 0:QC])
            nc.sync.dma_start(out=wk_sb[:, 0:4, :], in_=wk[:, 0:4, :])
            nc.sync.dma_start(out=wk_sb[:, 4:8, :], in_=wk[:, 4:8, :])
            nc.sync.dma_start(out=wv_sb, in_=wv[:, :, :])

            tri = sing.tile([128, 128], BF16, tag="tri")
            nc.vector.memset(tri, 1.0)
            nc.gpsimd.affine_select(
                out=tri, in_=tri, compare_op=mybir.AluOpType.is_ge,
                fill=0.0, base=0, channel_multiplier=-1, pattern=[[1, 128]])
            ones = sing.tile([1, 64], BF16, tag="ones")
            nc.vector.memset(ones, 1.0)

            bq_sb = sing.tile([128, 2], F32, tag="bq")
            bk_sb = sing.tile([128, 2], F32, tag="bk")
            for o in range(2):
                nc.sync.dma_start(out=bq_sb[:, o:o + 1], in_=bqr[o * 128:(o + 1) * 128, :])
                nc.sync.dma_start(out=bk_sb[:, o:o + 1], in_=bkr[o * 128:(o + 1) * 128, :])
            bv_sb = sing.tile([128, OC], F32, tag="bv")
            nc.sync.dma_start(out=bv_sb, in_=_bc(bvr[0:1, :], 128))
            bv4 = bv_sb.rearrange("p (h c) -> p h c", h=HPG)
            # prefetch the remaining x^T groups and wo behind the hot DMAs
            nc.sync.dma_start(out=xtg[1][:, 0:4, :], in_=xt[:, 0:4, QC:2 * QC])
            nc.sync.dma_start(out=xtg[1][:, 4:8, :], in_=xt[:, 4:8, QC:2 * QC])
            nc.sync.dma_start(out=wo_sb, in_=wo[:, :, :])
            nc.sync.dma_start(out=xtg[2], in_=xt[:, :, 2 * QC:3 * QC])
            nc.sync.dma_start(out=xtg[3], in_=xt[:, :, 3 * QC:4 * QC])

            qt = per.tile([128, 2, S], BF16, tag="qt", name="qt")
            kt = per.tile([128, 2, S], BF16, tag="kt", name="kt")
            ot = per.tile([128, 2, S], BF16, tag="ot", name="ot")
            v4 = per.tile([128, NT, HPG, VW], BF16, tag="v4", name="v4")

            def interleave(*lists):
                import heapq
                h, outl = [], []
                for li, L in enumerate(lists):
                    if L:
                        heapq.heappush(h, (0.0, li, 0))
                while h:
                    pos, li, idx = heapq.heappop(h)
                    outl.append(lists[li][idx])
                    if idx + 1 < len(lists[li]):
                        heapq.heappush(h, (pos + 1.0 / len(lists[li]), li, idx + 1))
                return outl

            def ab_items(g):
                """Projections for token group g (512 tokens)."""
                items = []
                qk_ps = {}

                def qk_chunk(wsb, bsb, dst, o, half):
                    def f():
                        if half == 0:
                            qk_ps[(id(wsb), o)] = pp.tile(
                                [128, QC], F32, tag="gp", name=f"qk{g}_{o}")
                        ps = qk_ps[(id(wsb), o)]
                        for kc in range(4 * half, 4 * half + 4):
                            nc.tensor.matmul(
                                ps,
                                lhsT=wsb[:, kc, o * 128:(o + 1) * 128],
                                rhs=xtg[g][:, kc, :],
                                start=(kc == 0), stop=(kc == NM - 1))
                        if half == 1:
                            nc.vector.tensor_scalar_add(
                                out=dst[:, o, g * QC:(g + 1) * QC],
                                in0=ps, scalar1=bsb[:, o:o + 1])
                    return f
                for wsb, bsb, dst in ((wq_sb, bq_sb, qt), (wk_sb, bk_sb, kt)):
                    for half in range(2):
                        for o in range(2):
                            items.append(qk_chunk(wsb, bsb, dst, o, half))

                def v_chunk(tp):
                    # two token tiles (tp, tp+1) share one [128, 512] psum
                    def f():
                        pv = pp.tile([128, 2, OC], F32, tag="gp", name=f"pv{tp}")
                        for i in range(2):
                            tt = 4 * g + tp + i
                            for kc in range(NM):
                                nc.tensor.matmul(
                                    pv[:, i, :],
                                    lhsT=xtg[g][:, kc, (tp + i) * 128:(tp + i + 1) * 128],
                                    rhs=wv_sb[:, kc, :],
                                    start=(kc == 0), stop=(kc == NM - 1))
                        for i in range(2):
                            tt = 4 * g + tp + i
                            nc.vector.tensor_add(
                                out=v4[:, tt, :, 0:DH],
                                in0=pv[:, i, :].rearrange("p (h c) -> p h c", h=HPG),
                                in1=bv4)
                            nc.gpsimd.memset(v4[:, tt, :, DH:VW], 1.0)
                    return f
                for tp in (0, 2):
                    items.append(v_chunk(tp))
                return items

            c_state = {}

            def c_items(qc, heads=tuple(range(HPG))):
                """Attention for q chunk qc (512 queries), given heads."""
                items = []
                nkt = 4 * qc + 4
                pavs = c_state.setdefault(qc, {})

                def pair_step(h, ktp):
                    o, r = h // 2, (h % 2) * 64
                    def f():
                        qt_h = qt[r:r + 64, o, :]
                        kt_h = kt[r:r + 64, o, :]
                        if ktp == 0:
                            pavs[h] = pp.tile([VW, QC], F32, tag="pav",
                                              name=f"pav{qc}_{h}")
                        pav = pavs[h]
                        kts = [k for k in (ktp, ktp + 1) if k < nkt]
                        ps = pp.tile([128, 1024], F32, tag="ps",
                                     name=f"ps{qc}_{h}_{ktp}")
                        offs = [max(k * 128 - qc * QC, 0) for k in kts]
                        for i, k in enumerate(kts):
                            nc.tensor.matmul(
                                ps[:, i * 512 + offs[i]:(i + 1) * 512],
                                lhsT=kt_h[:, k * 128:(k + 1) * 128],
                                rhs=qt_h[:, qc * QC + offs[i]:(qc + 1) * QC],
                                start=True, stop=True)
                        at = apool.tile([128, 1024], BF16, tag="at",
                                        name=f"at{qc}_{h}_{ktp}")
                        if offs[0] == offs[-1]:
                            # same offset: one exp instruction for the pair
                            w = 512 * len(kts)
                            nc.scalar.activation(
                                out=at[:, offs[0]:w], in_=ps[:, offs[0]:w],
                                func=mybir.ActivationFunctionType.Exp,
                                scale=1.0 / 8.0)
                        else:
                            for i, k in enumerate(kts):
                                nc.scalar.activation(
                                    out=at[:, i * 512 + offs[i]:(i + 1) * 512],
                                    in_=ps[:, i * 512 + offs[i]:(i + 1) * 512],
                                    func=mybir.ActivationFunctionType.Exp,
                                    scale=1.0 / 8.0)
                        for i, k in enumerate(kts):
                            off = offs[i]
                            if k * 128 - qc * QC >= 0:
                                nc.gpsimd.tensor_mul(
                                    out=at[:, i * 512 + off:i * 512 + off + 128],
                                    in0=at[:, i * 512 + off:i * 512 + off + 128],
                                    in1=tri)
                            nc.tensor.matmul(
                                pav[:, off:QC],
                                lhsT=v4[:, k, h, :],
                                rhs=at[:, i * 512 + off:(i + 1) * 512],
                                start=(k == 0), stop=(k == nkt - 1))
                    return f

                recs = {}

                def norm_recip(h):
                    def f():
                        rec = rpool.tile([1, QC], BF16, tag="rec",
                                         name=f"rec{qc}_{h}")
                        with nc.allow_low_precision(
                                reason="1/l broadcast fed through bf16 PE "
                                       "outer product; 0.4% on the "
                                       "normalizer is within budget"):
                            nc.vector.reciprocal(out=rec, in_=pavs[h][DH:VW, :])
                        recs[h] = rec
                    return f

                def norm_apply(h):
                    # PE outer-product broadcast of 1/l; deferred into the
                    # next head's instruction stream so the in-order PE queue
                    # has score work in front of it while DVE computes rec.
                    o, r = h // 2, (h % 2) * 64
                    def f():
                        rbt = pp.tile([128, QC], F32, tag="gp",
                                      name=f"rbp{qc}_{h}")
                        rbp = rbt[0:64, :]
                        nc.tensor.matmul(rbp, lhsT=ones, rhs=recs[h],
                                         start=True, stop=True)
                        rb = rpool.tile([64, QC], F32, tag="rb",
                                        name=f"rb{qc}_{h}")
                        nc.vector.tensor_copy(out=rb, in_=rbp)
                        nc.vector.tensor_mul(
                            out=ot[r:r + 64, o, qc * QC:(qc + 1) * QC],
                            in0=pavs[h][0:DH, :], in1=rb)
                    return f

                pend = None
                for h in heads:
                    npair = (nkt + 1) // 2
                    for j, ktp in enumerate(range(0, nkt, 2)):
                        items.append(pair_step(h, ktp))
                        if j == min(2, npair - 1) and pend is not None:
                            items.append(pend)
                            pend = None
                    items.append(norm_recip(h))
                    pend = norm_apply(h)
                items.append(pend)
                return items

            def d_items(g, fin=False):
                """Output projection for token tiles 4g..4g+3."""
                items = []

                def out_tile(tt):
                    def f():
                        ob = opool.tile([128, D], F32, tag="ob", name=f"ob{tt}")
                        for nb in range(2):
                            po = pp.tile([128, 512], F32, tag="gp",
                                         name=f"po{tt}_{nb}")
                            for cb in range(2):
                                nc.tensor.matmul(
                                    po,
                                    lhsT=ot[:, cb, tt * 128:(tt + 1) * 128],
                                    rhs=wo_sb[:, cb, nb * 512:(nb + 1) * 512],
                                    start=(cb == 0), stop=(cb == 1))
                            dst = ob[:, nb * 512:(nb + 1) * 512]
                            if fin and (tt + nb) % 2 == 1:
                                nc.scalar.copy(out=dst, in_=po)
                            else:
                                nc.vector.tensor_copy(out=dst, in_=po)
                            nc.sync.dma_start(
                                out=out[tt * 128:(tt + 1) * 128,
                                        nb * 512:(nb + 1) * 512],
                                in_=dst)
                    return f
                for tt in range(4 * g, 4 * g + 4):
                    items.append(out_tile(tt))
                return items

            # warm up the tensor engine while the first DMAs land: the
            # p-state model needs ~3us of continuous execution to reach
            # full clock, and the first real matmul can't start before the
            # wq/x^T transfers complete (~4us).
            warm = pp.tile([128, 1024], F32, tag="ps", name="warm")
            for i in range(30):
                nc.tensor.matmul(warm[:, 0:128], lhsT=tri, rhs=tri,
                                 start=True, stop=True)

            # Rounds are balanced so per-round ACT (exp) work stays below
            # per-round PE work; qc=3's first two heads are pulled into
            # round 3 because round 4 would otherwise be exp-bound.
            for f in ab_items(0):
                f()
            for f in interleave(c_items(0), ab_items(1)):
                f()
            for f in interleave(c_items(1), ab_items(2)):
                f()
            for f in interleave(c_items(2), ab_items(3)):
                f()
            # the last attention chunk is exp-bound on ACT, so all the
            # output-projection work is saved up as PE filler for it
            d1 = d_items(1)
            for f in interleave(c_items(3, heads=(0, 1)),
                                d_items(0) + d1[:1]):
                f()
            for f in interleave(c_items(3, heads=(2, 3)),
                                d1[1:] + d_items(2)):
                f()
            for f in d_items(NQC - 1, fin=True):
                f()
    return nc


def _get_nc():
    key = "v2"
    if key not in _NC_CACHE:
        _patch_tile_drain()
        _patch_compile_hook()
        _NC_CACHE[key] = build_nc()
    return _NC_CACHE[key]


def make_in_maps(inputs):
    x = np.asarray(inputs["x"], dtype=np.float32)
    Wq = np.asarray(inputs["Wq"], dtype=np.float32)
    Wk = np.asarray(inputs["Wk"], dtype=np.float32)
    Wv = np.asarray(inputs["Wv"], dtype=np.float32)
    Wo = np.asarray(inputs["Wo"], dtype=np.float32)
    bq = np.asarray(inputs["bq"], dtype=np.float32)
    bk = np.asarray(inputs["bk"], dtype=np.float32)
    bv = np.asarray(inputs["bv"], dtype=np.float32)

    def chunked_w(w):
        # [D, OC] -> [128, NM, OC] bf16, d-chunk-major on partitions
        return np.ascontiguousarray(
            w.reshape(NM, 128, -1).transpose(1, 0, 2).astype(NPBF))

    xts = []
    for b in range(B):
        xts.append(np.ascontiguousarray(
            x[b].T.reshape(NM, 128, S).transpose(1, 0, 2).astype(NPBF)))

    in_maps = []
    for c in range(8):
        b, g = c // 4, c % 4
        cols = slice(g * OC, (g + 1) * OC)
        in_maps.append({
            "xt": xts[b],
            "wq": chunked_w(Wq[:, cols]),
            "wk": chunked_w(Wk[:, cols]),
            "wv": chunked_w(Wv[:, cols]),
            "wo": np.ascontiguousarray(
                Wo[cols, :].reshape(2, 128, D).transpose(1, 0, 2).astype(NPBF)),
            "bq": np.ascontiguousarray(bq[cols]),
            "bk": np.ascontiguousarray(bk[cols]),
            "bv": np.ascontiguousarray(bv[cols]),
        })
    return in_maps


def combine(results, inputs):
    bo = np.asarray(inputs["bo"], dtype=np.float32)
    out = np.zeros((B, S, D), dtype=np.float32)
    for c in range(8):
        out[c // 4] += np.asarray(results[c]["out"], dtype=np.float32)
    out += bo[None, None, :]
    return out


def kernel(**inputs) -> np.ndarray:
    nc = _get_nc()
    in_maps = make_in_maps(inputs)
    res = run_bass_kernel_spmd(nc, in_maps, core_ids=list(range(8)))
    return combine(res.results, inputs)


if __name__ == "__main__":
    import jax
    print(jax.devices())
